# revision 54
# baseline (speedup 1.0000x reference)
"""Nearest-neighbor tokenizer on Trainium2: 8 NeuronCores, code-sharded.

Per token x (d=512) against codebook C [16384, 512]:
    dist^2(x,c) = ||x||^2 + ||c||^2 - 2 x.c
    id = argmin_c dist^2   if min_c dist^2 <= 900 else -1

v7 architecture (fp8 DoubleRow candidate search, exact host rescore):
  - Shard by CODES: core g owns codes[g*2048:(g+1)*2048], sees all 8192
    tokens (64 token tiles of 128).
  - Device ranks codes by v = x.c + b where b = 256 - ||c||^2/2 (the
    global +256 shift is rank-neutral). GEMM runs in fp8e4m3 with
    perf_mode=DoubleRow (K=256 per matmul, 0.5 cycles/row in the cost
    model): per tile 4 PSUM slices x [bias MM (4-term greedy fp8
    decomposition of b, K=4) + 2 main MMs] -> ps [128, 2048] f32.
  - PSUM exit is the bottleneck (only ACT and DVE can read PSUM, 1
    elem/cycle each, and coupling a DVE psum-reader into the psum-free
    path stalls the PE on the DVE completion counter). So: ONE ACT
    activation-copy drains the whole tile (2048 f32 -> bf16 h,
    1892 ns, ACT runs back-to-back = the steady-state period), and the
    DVE runs a bf16 tensor_tensor pair-max tree (2x mode)
    1024/512/256/128/64 -> 64 slots of 32 codes, then max8/max_index
    over the 64 slot values -> top-8 (value, slot) per (token, core).
  - Host keeps slots within MARGIN of the global best value and
    rescores their codes exactly in float64; argmin + threshold then
    reproduce the reference (the fp8 ranking noise is ~0.9 sigma,
    MARGIN=10 is ~8 sigma; CoreSim-measured headroom is 4.0).
"""

import sys

import numpy as np

try:
    import concourse.bass as _probe_bass  # noqa: F401
except Exception:  # pragma: no cover
    sys.path.insert(0, "/opt/trn_rl_repo")

import ml_dtypes

B, S, D = 4, 2048, 512
C = 16384
N_CORES = 8
NTOK = B * S                   # 8192 tokens, all seen by every core
N_TILES = NTOK // 128          # 64 token tiles
G = C // N_CORES               # 2048 codes per core
XCH = 8                        # token tiles per xs DMA chunk
MARGIN = 10.0
NSLOT = 64                     # tree slots of 32 codes each
HFO = 48 * 8                   # output chunk boundaries (tiles 48, 56)
HF2 = 56 * 8

FP8 = ml_dtypes.float8_e4m3

_CACHE: dict = {}


def _build_program(nc=None):
    import concourse.tile as tile
    from concourse import mybir

    f32 = mybir.dt.float32
    bf16 = mybir.dt.bfloat16
    fp8 = mybir.dt.float8e4
    u32 = mybir.dt.uint32
    Alu = mybir.AluOpType
    Act = mybir.ActivationFunctionType
    DR = mybir.MatmulPerfMode.DoubleRow

    if nc is None:
        from concourse import bacc

        nc = bacc.Bacc("TRN2", target_bir_lowering=False, debug=False)

    xs_d = nc.declare_dram_parameter("xs", [128, N_TILES * 512], fp8, isOutput=False)
    cr_d = nc.declare_dram_parameter("cr", [128, 4 * G], fp8, isOutput=False)
    cb_d = nc.declare_dram_parameter("cb", [2, 2 * G + 256], fp8, isOutput=False)
    cval_d = nc.declare_dram_parameter("cval", [128, N_TILES * 8], f32, isOutput=True)
    cidx_d = nc.declare_dram_parameter("cidx", [128, N_TILES * 8], u32, isOutput=True)

    with tile.TileContext(nc) as tc:
        with (
            tc.tile_pool(name="const", bufs=1) as const,
            tc.tile_pool(name="xch", bufs=2) as xch,
            tc.tile_pool(name="work", bufs=2) as work,
            tc.tile_pool(name="psum", bufs=2, space="PSUM") as psum,
        ):
            # cb+on packed in one small DMA (Pool SWDGE queue); cr arrives
            # as four per-slice tiles so slice s only waits its own chunk
            # (cr0 on SP ahead of xs chunk 0, cr1-3 on the ACT queue).
            cr_v = cr_d[:].rearrange("p (m i c) -> p m i c", m=2, i=2)
            crs = []
            for s in range(4):
                crt = const.tile([128, 2, 2, 512], fp8, name=f"cr{s}")
                eng = nc.sync if s == 0 else nc.scalar
                eng.dma_start(crt[:], cr_v[:, :, :, s * 512:(s + 1) * 512])
                crs.append(crt)
            cbon = const.tile([2, 2 * G + 256], fp8, name="cbon")
            nc.sync.dma_start(cbon[:], cb_d[:])
            cb = cbon[:, 0:2 * G].rearrange("p (i c) -> p i c", i=2)
            on = cbon[:, 2 * G:].rearrange("p (i c) -> p i c", i=2)

            cval = const.tile([128, N_TILES * 8], f32, name="cval")
            cidx = const.tile([128, N_TILES * 8], u32, name="cidx")

            # preload the ACT function table off the critical path
            warm = const.tile([2, 8], f32, name="warm")
            nc.vector.memset(warm[:], 0.0)
            warmo = const.tile([2, 8], bf16, name="warmo")
            nc.scalar.activation(warmo[:], warm[:], Act.Copy)

            # warm the PE p-state ramp (needs ~3us of continuous matmuls
            # before full clock) using memset inputs - no DMA dependency
    
            wl = const.tile([2, 2, 128], fp8, name="wl")
            nc.vector.memset(wl[:], 1.0)
            wr = const.tile([2, 2, 512], fp8, name="wr")
            nc.gpsimd.memset(wr[:], 1.0)
            wps = psum.tile([128, 512], f32, name="wps", tag="pst")
            for w in range(8):
                nc.tensor.matmul(
                    wps[:], wl[:], wr[:],
                    start=(w == 0), stop=(w == 7), perf_mode=DR,
                )
            wdr = work.tile([128, 8], bf16, name="wdr")
            nc.vector.tensor_reduce(
                wdr[:], wps[:].rearrange("p (a b) -> p a b", b=64),
                mybir.AxisListType.X, Alu.max,
            )

            for c8 in range(N_TILES // XCH):
                xc = xch.tile([128, XCH, 2, 2, 128], fp8, name="xc")
                if c8 == 0:
                    nc.sync.dma_start(xc[:, 0:2], xs_d[:, 0:1024])
                    nc.sync.dma_start(xc[:, 2:XCH], xs_d[:, 1024:XCH * 512])
                else:
                    nc.sync.dma_start(
                        xc[:], xs_d[:, c8 * XCH * 512:(c8 + 1) * XCH * 512]
                    )
                for k in range(XCH):
                    t = c8 * XCH + k
                    ps = psum.tile([128, G], f32, name="ps", tag="pst")
                    for s in range(4):
                        out = ps[:, s * 512:(s + 1) * 512]
                        nc.tensor.matmul(
                            out, on, cb[:, :, s * 512:(s + 1) * 512],
                            start=True, stop=False, perf_mode=DR,
                        )
                        for m in range(2):
                            nc.tensor.matmul(
                                out, xc[:, k, m],
                                crs[s][:, m],
                                start=False, stop=(m == 1), perf_mode=DR,
                            )

                    mfin = work.tile([128, NSLOT], bf16, name="mfin")
                    h = work.tile([128, G], bf16, name="h")
                    nc.scalar.activation(h[:], ps[:], Act.Copy)
                    t1 = work.tile([128, 1024], bf16, name="t1")
                    nc.vector.tensor_tensor(
                        t1[:], h[:, 0:1024], h[:, 1024:2048], Alu.max
                    )
                    t2 = work.tile([128, 512], bf16, name="t2")
                    nc.vector.tensor_tensor(
                        t2[:], t1[:, 0:512], t1[:, 512:1024], Alu.max
                    )
                    t3 = work.tile([128, 256], bf16, name="t3")
                    nc.vector.tensor_tensor(
                        t3[:], t2[:, 0:256], t2[:, 256:512], Alu.max
                    )
                    t4 = work.tile([128, 128], bf16, name="t4")
                    nc.vector.tensor_tensor(
                        t4[:], t3[:, 0:128], t3[:, 128:256], Alu.max
                    )
                    nc.vector.tensor_tensor(
                        mfin[:], t4[:, 0:64], t4[:, 64:128], Alu.max
                    )
                    nc.vector.max(cval[:, t * 8:(t + 1) * 8], mfin[:])
                    nc.vector.max_index(
                        cidx[:, t * 8:(t + 1) * 8], cval[:, t * 8:(t + 1) * 8],
                        mfin[:],
                    )
                if c8 == 5:
                    nc.sync.dma_start(cval_d[:, 0:HFO], cval[:, 0:HFO])
                    nc.gpsimd.dma_start(cidx_d[:, 0:HFO], cidx[:, 0:HFO])
                if c8 == 6:
                    nc.sync.dma_start(cval_d[:, HFO:HF2], cval[:, HFO:HF2])
                    nc.gpsimd.dma_start(cidx_d[:, HFO:HF2], cidx[:, HFO:HF2])

            nc.sync.dma_start(cval_d[:, HF2:], cval[:, HF2:])
            nc.scalar.dma_start(cidx_d[:, HF2:], cidx[:, HF2:])

    return nc


def _slot_cols() -> list:
    """slot j -> np.array of tile-local psum columns (code ids within the
    core's 2048-code shard)."""
    lvl = [np.array([i, i + 1024]) for i in range(1024)]                # t1
    lvl = [np.concatenate([lvl[i], lvl[i + 512]]) for i in range(512)]  # t2
    lvl = [np.concatenate([lvl[i], lvl[i + 256]]) for i in range(256)]  # t3
    lvl = [np.concatenate([lvl[i], lvl[i + 128]]) for i in range(128)]  # t4
    lvl = [np.concatenate([lvl[i], lvl[i + 64]]) for i in range(64)]    # mfin
    return [np.sort(lvl[k]) for k in range(NSLOT)]


def _prepare_in_maps(x: np.ndarray, codes: np.ndarray) -> list:
    x = np.ascontiguousarray(np.asarray(x, dtype=np.float32).reshape(NTOK, D))
    codes = np.ascontiguousarray(np.asarray(codes, dtype=np.float32))
    x8 = x.astype(FP8)
    c8 = codes.astype(FP8)

    # xs[p, t, m, i, tok] = x8[t*128+tok, m*256+i*128+p]
    xs = np.ascontiguousarray(
        x8.reshape(N_TILES, 128, 2, 2, 128).transpose(4, 0, 2, 3, 1)
    ).reshape(128, -1)
    on = np.ones((2, 256), dtype=FP8)

    in_maps = []
    for g in range(N_CORES):
        cg8 = c8[g * G:(g + 1) * G]          # [2048, 512] fp8
        # cr[p, m, i, c] = cg8[c, m*256+i*128+p]
        cr = np.ascontiguousarray(
            cg8.reshape(G, 2, 2, 128).transpose(3, 1, 2, 0)
        ).reshape(128, -1)
        cg64 = codes[g * G:(g + 1) * G].astype(np.float64)
        b = 256.0 - 0.5 * (cg64 ** 2).sum(1)  # [-64, 64]-ish, rank-neutral shift
        terms = []
        r = b.copy()
        for _ in range(4):
            tq = r.astype(FP8)
            terms.append(tq)
            r = r - tq.astype(np.float64)
        cb = np.stack(terms).reshape(2, 2 * G)
        cb = np.ascontiguousarray(np.concatenate([cb, on], axis=1))
        in_maps.append({"xs": xs, "cr": cr, "cb": cb})
    return in_maps


def _postprocess(results: list, x: np.ndarray, codes: np.ndarray) -> np.ndarray:
    x64 = np.asarray(x, dtype=np.float64).reshape(NTOK, D)
    c64 = np.asarray(codes, dtype=np.float64)
    c2 = (c64 ** 2).sum(1)
    x2 = (x64 ** 2).sum(1)

    # [NTOK, N_CORES, 8] top-8 slot values / slot ids per core
    vals = np.empty((NTOK, N_CORES, 8), np.float64)
    slots = np.empty((NTOK, N_CORES, 8), np.int64)
    for g in range(N_CORES):
        cv = np.asarray(results[g]["cval"], np.float64)
        ci = np.asarray(results[g]["cidx"]).astype(np.int64)
        vals[:, g, :] = cv.reshape(128, N_TILES, 8).transpose(1, 0, 2).reshape(NTOK, 8)
        slots[:, g, :] = ci.reshape(128, N_TILES, 8).transpose(1, 0, 2).reshape(NTOK, 8)

    best = vals.reshape(NTOK, -1).max(1)
    keep = vals >= (best[:, None, None] - MARGIN)
    tk, gk, rk = np.nonzero(keep)
    sk = slots[tk, gk, rk]

    slot_cols = _slot_cols()                 # NSLOT x 32 cols
    ids = np.full(NTOK, -1, np.int64)
    bestd = np.full(NTOK, np.inf, np.float64)

    cmap = np.stack(slot_cols)               # [NSLOT, 32]
    cand = cmap[sk] + (gk * G)[:, None]      # [K, 32]
    CH = 65536
    for i in range(0, len(tk), CH):
            tc_ = tk[i:i + CH]
            cc = cand[i:i + CH]
            xc = np.einsum("kcd,kd->kc", c64[cc], x64[tc_], optimize=True)
            d2 = np.maximum(x2[tc_][:, None] + c2[cc] - 2.0 * xc, 0.0)
            # fold into per-token running argmin with lowest-id tie-break
            tflat = np.repeat(tc_, cc.shape[1])
            dflat = d2.ravel()
            cflat = cc.ravel()
            order = np.lexsort((cflat, dflat, tflat))
            to, do_, co = tflat[order], dflat[order], cflat[order]
            first = np.unique(to, return_index=True)[1]
            tsel, dsel, csel = to[first], do_[first], co[first]
            upd = (dsel < bestd[tsel]) | (
                (dsel == bestd[tsel]) & (csel < ids[tsel])
            )
            bestd[tsel[upd]] = dsel[upd]
            ids[tsel[upd]] = csel[upd]

    ids = np.where(bestd <= 900.0, ids, -1)
    return ids.reshape(B, S).astype(np.int32)


def kernel(x: np.ndarray, codes: np.ndarray) -> np.ndarray:
    from concourse.bass_utils import run_bass_kernel_spmd

    if "nc" not in _CACHE:
        nc = _build_program()
        nc.finalize()
        _CACHE["nc"] = nc
    in_maps = _prepare_in_maps(x, codes)
    res = run_bass_kernel_spmd(_CACHE["nc"], in_maps, list(range(N_CORES)))
    return _postprocess(res.results, x, codes)
